# revision 22
# baseline (speedup 1.0000x reference)
"""Trainium2 Bass kernel for nn_AttentionBlock (B=2, C=512, L=64x64, 8 heads).

Sharding v2: 8 cores = 2 (batch) x 4 (query-blocks of 1024 columns).
Each core computes group-norm + k,v for ALL 8 heads (redundant across the
4 cores of a batch), q/attention/proj only for its own 1024 query columns,
and writes a [512, 1024] fp32 output slice.  NO collectives -- the host
concatenates the L-slices.  This trades ~40us of redundant k/v PE work
for the old AllGather + inter-core sync (~300us).

The host ROTATES x along L per core (np.roll by -1024*qb) so each core's
query block sits at local columns 0-1023; attention and group-norm are
permutation-invariant over keys/columns, so this is exact.

Heads are processed as 4 "pairs" (2 heads per 128 partitions); virtual
t-blocks are (tb, pair), tb in {0,1}.  Within a vtb the flat order is
head-major (head A s0..s31, then head B) so consecutive s-tiles pair up
for fp8 DoubleRow AV matmuls (2 s-tiles per 512-cycle pass).  exp is
emitted with input bias -2 so softmax weights fit fp8e4m3's 240 max
(uniform scaling cancels in the softmax).  Each head finalizes as soon
as its s=31 AV lands, freeing its PSUM accumulator mid-vtb.

PSUM budget (8 banks): S^T groups 2x3 banks (stp tag "st"), AV
accumulators 2x1 (accp tag "aps").  qkv/vT producer matmuls borrow "st"
rotation slots so they never contend with live AV accumulators; proj
runs after the attention pools close, on its own 4-bank pool.

DMA routing: x chunks + vT ones + xres on the SP HWDGE queue; weights /
biases / small constants on the ACT HWDGE queue, so the group-norm
stats (gated on x) never wait behind weight descriptors.

Matmul dtypes: bf16 S^T / qkv / proj, fp8e4m3 AV (DoubleRow where s-tiles
pair), fp32 group-norm matmuls, float32r 1/denom broadcast.
"""

import sys

if "/opt/trn_rl_repo" not in sys.path:
    sys.path.insert(0, "/opt/trn_rl_repo")

import numpy as np
import ml_dtypes

B, C = 2, 512
HW_L = 4096          # 64*64
NH, CHD, NG = 8, 64, 32
EPS = 1e-5
N_CORES = 8
SCALE = 1.0 / np.sqrt(np.sqrt(CHD))
QBLK = 1024          # query columns per core
TBLK = 256           # t-block (query block) size
NTB = QBLK // TBLK   # t-blocks per pair (4)
NPAIR = 4            # head pairs
KT = C // 128        # 4 input-channel tiles
GSZ = 6              # exp group size (s-tiles per [128,GSZ,TBLK] psum tile)
EGB = 14             # exp-tile pool bufs
PRE = 8              # S^T/exp groups emitted ahead of their AV
FP8AV = True         # fp8e4m3 AV with DoubleRow pairing
EXP_BIAS = -2.0      # exp(s-2): keeps weights under fp8e4m3 max (240)
REPS = 1             # emit the whole body N times (differential timing)


def build_nc(L=HW_L, reps=None):
    import concourse.bass as bass
    import concourse.tile as tile
    from concourse import bacc, mybir
    from contextlib import ExitStack

    f32 = mybir.dt.float32
    f32r = mybir.dt.float32r
    bf16 = mybir.dt.bfloat16
    f8 = mybir.dt.float8e4
    avdt = f8 if FP8AV else bf16
    AF = mybir.ActivationFunctionType
    OP = mybir.AluOpType
    AX = mybir.AxisListType
    DR = mybir.MatmulPerfMode.DoubleRow

    def R(ap):
        # reinterpret fp32 operand as float32r: 1 PE cycle/row (vs 4 for
        # fp32) when the output free size is >= 256
        return ap.bitcast(f32r)

    NS = L // 128           # 32 key s-tiles per pair
    NCH = L // 512          # 8 512-wide chunks of L
    VPAD = 80 if FP8AV else 65   # vT inner stride (DoubleRow needs %16==0)

    nc = bacc.Bacc("TRN2", target_bir_lowering=False, debug=False,
                   num_devices=N_CORES)

    x_ext = nc.dram_tensor("x", [C, L], bf16, kind="ExternalInput")
    xres_ext = nc.dram_tensor("xres", [C, QBLK], f32, kind="ExternalInput")
    wq_ext = nc.dram_tensor("wqT", [C, C], bf16, kind="ExternalInput")
    wk_ext = nc.dram_tensor("wkT", [C, C], bf16, kind="ExternalInput")
    wv_ext = nc.dram_tensor("wvT", [C, C], bf16, kind="ExternalInput")
    pw_ext = nc.dram_tensor("pwT", [C, C], bf16, kind="ExternalInput")
    bq_ext = nc.dram_tensor("bq", [128, NPAIR], f32, kind="ExternalInput")
    bk_ext = nc.dram_tensor("bk", [128, NPAIR], f32, kind="ExternalInput")
    pb_ext = nc.dram_tensor("pb", [128, KT], f32, kind="ExternalInput")
    bvr_ext = nc.dram_tensor("bvr", [1, C], bf16, kind="ExternalInput")
    nw_ext = nc.dram_tensor("nw", [C, 1], f32, kind="ExternalInput")
    nb_ext = nc.dram_tensor("nb", [C, 1], f32, kind="ExternalInput")
    gi_ext = nc.dram_tensor("gind", [NG, C], f32, kind="ExternalInput")
    giT_ext = nc.dram_tensor("gindT", [C, NG], f32, kind="ExternalInput")
    ones8_ext = nc.dram_tensor("ones8", [128, 64], avdt,
                               kind="ExternalInput")
    onesf_ext = nc.dram_tensor("onesf", [128, 64], f32, kind="ExternalInput")
    onesr_ext = nc.dram_tensor("onesr", [1, 128], bf16, kind="ExternalInput")
    out_ext = nc.dram_tensor("out", [C, QBLK], f32, kind="ExternalOutput")

    with tile.TileContext(nc, num_cores=N_CORES) as tc, ExitStack() as ctx:
        pers = ctx.enter_context(tc.tile_pool(name="pers", bufs=1))
        # accp: 1 bank -- the packed per-vtb AV accumulator [65, 2, TBLK]
        # (both heads side by side) plus the small group-norm matmuls
        accp = ctx.enter_context(
            tc.tile_pool(name="accp", bufs=1, space="PSUM"))

        # ---- persistent tiles (loads deferred until after the x DMAs so
        # the group-norm stats never wait behind weight descriptors on the
        # shared HWDGE) ----------------------------------------------------
        # weight layout: w*_all[:, m, :] holds rows 128m..128m+127 of the
        # [C, C] transposed weight; stationary AP for (m, pair j) is
        # w*_all[:, m, 128j:128j+128].
        wq_all = pers.tile([128, KT, C], bf16, tag="wq")
        wk_all = pers.tile([128, KT, C], bf16, tag="wk")
        wv_all = pers.tile([128, KT, C], bf16, tag="wv")
        pw_all = pers.tile([128, KT, C], bf16, tag="pw")
        bq_sb = pers.tile([128, NPAIR], f32, tag="bq")
        bk_sb = pers.tile([128, NPAIR], f32, tag="bk")
        pb_sb = pers.tile([128, KT], f32, tag="pb")
        w_part = [pers.tile([128, 1], f32, tag=f"nw{m}", name=f"nw{m}")
                  for m in range(KT)]
        b_part = [pers.tile([128, 1], f32, tag=f"nb{m}", name=f"nb{m}")
                  for m in range(KT)]
        gi_sb = pers.tile([NG, C], f32, tag="gi")
        giT_sb = [pers.tile([128, NG], f32, tag=f"giT{m}", name=f"giT{m}")
                  for m in range(KT)]
        ones_sb = pers.tile([128, 64], f32, tag="ones")
        onesr_sb = pers.tile([1, 128], bf16, tag="onesr")
        bvr_sb = pers.tile([1, C], bf16, tag="bvr")

        def emit_pers_loads():
            # ordered by first use: group-norm stats path, then k/q weights
            # for the bootstrap, then the rest
            nc.gpsimd.dma_start(gi_sb[:], gi_ext[:])
            for m in range(KT):
                nc.gpsimd.dma_start(giT_sb[m][:],
                                    giT_ext[128 * m:128 * (m + 1), :])
                nc.gpsimd.dma_start(w_part[m][:],
                                    nw_ext[128 * m:128 * (m + 1), :])
                nc.gpsimd.dma_start(b_part[m][:],
                                    nb_ext[128 * m:128 * (m + 1), :])
            for m in range(KT):
                nc.gpsimd.dma_start(wk_all[:, m, :],
                                    wk_ext[128 * m:128 * (m + 1), :])
            nc.gpsimd.dma_start(bk_sb[:], bk_ext[:])
            for m in range(KT):
                nc.gpsimd.dma_start(wq_all[:, m, :],
                                    wq_ext[128 * m:128 * (m + 1), :])
            nc.gpsimd.dma_start(bq_sb[:], bq_ext[:])
            nc.gpsimd.dma_start(R(ones_sb[:]), R(onesf_ext[:]))
            nc.gpsimd.dma_start(onesr_sb[:], onesr_ext[:])
            nc.gpsimd.dma_start(bvr_sb[:], bvr_ext[:])
            for m in range(KT):
                nc.gpsimd.dma_start(wv_all[:, m, :],
                                    wv_ext[128 * m:128 * (m + 1), :])
            for j in range(NPAIR):
                nc.gpsimd.dma_start(vT_sb[:, j, :, :, 64:65],
                                    ones8_ext[:, 0:2 * NS])
            for m in range(KT):
                nc.gpsimd.dma_start(pw_all[:, m, :],
                                    pw_ext[128 * m:128 * (m + 1), :])
            nc.gpsimd.dma_start(pb_sb[:], pb_ext[:])

        # persistent activation tensors
        q_sb = pers.tile([128, NPAIR, QBLK], bf16, tag="q")
        k_sb = pers.tile([128, NPAIR, L], bf16, tag="k")
        # vT laid out [keys, pair, s-tile, head, VPAD] -- col 64 of each
        # head is the all-ones column accumulating the softmax denominator
        vT_sb = pers.tile([128, NPAIR, NS, 2, VPAD], avdt, tag="vT")
        a_sb = [pers.tile([128, QBLK], bf16, tag=f"a{m}", name=f"a{m}")
                for m in range(KT)]

        def emit_body(rep):
          with (
            tc.tile_pool(name=f"xpool{rep}", bufs=1) as xpool,
            tc.tile_pool(name=f"attn{rep}", bufs=1) as attnp,
          ):
            xres_t = [xpool.tile([128, QBLK], f32, tag=f"xr{m}",
                                 name=f"xr{m}") for m in range(KT)]
            with (
              tc.tile_pool(name=f"stp{rep}", bufs=2, space="PSUM") as stp,
              tc.tile_pool(name=f"prp{rep}",
                           bufs=(2 if TBLK == 512 else 1),
                           space="PSUM") as prp,
              tc.tile_pool(name=f"expp{rep}", bufs=EGB) as expp,
            ):
              xs = [xpool.tile([128, L], bf16, tag=f"x{m}", name=f"x{m}")
                    for m in range(KT)]
              stats = [xpool.tile([128, L // 512, 6], f32, tag=f"bs{m}",
                                  name=f"bs{m}") for m in range(KT)]
              mv = [xpool.tile([128, 2], f32, tag=f"mv{m}", name=f"mv{m}")
                    for m in range(KT)]
              rhs_m = [xpool.tile([128, 2], f32, tag=f"rh{m}", name=f"rh{m}")
                       for m in range(KT)]
              # x in 8 half-row DMAs (fewer descriptors on the shared
              # HWDGE).  m=3 loads first and its channel stats run on the
              # (otherwise idle) ACT engine via activation accum_out:
              # Copy(x/4096) accumulates the mean, Square(x/64) accumulates
              # E[x^2] (= sum(x^2)/4096).  m=0..2 use DVE bn_stats per
              # 512-chunk as each half lands.
              HHALF = L // 2
              acc3 = xpool.tile([128, 2, L // 512], f32, tag="acc3")
              ascr = [xpool.tile([128, 512], bf16, tag=f"ascr{z}",
                                 name=f"ascr{z}") for z in range(2)]
              for m in [3, 0, 1, 2]:
                  for hh in range(2):
                      nc.sync.dma_start(
                          xs[m][:, HHALF * hh:HHALF * (hh + 1)],
                          x_ext[128 * m:128 * (m + 1),
                                HHALF * hh:HHALF * (hh + 1)])
                      if m == 3 and hh == 0 and rep == 0:
                          emit_pers_loads()
                      for i in range(4 * hh, 4 * (hh + 1)):
                          if m == 3:
                              sl = xs[m][:, 512 * i:512 * (i + 1)]
                              nc.scalar.activation(
                                  ascr[0][:], sl, AF.Copy,
                                  scale=1.0 / 4096.0,
                                  accum_out=acc3[:, 0, i:i + 1])
                              nc.scalar.activation(
                                  ascr[1][:], sl, AF.Square,
                                  scale=1.0 / 64.0,
                                  accum_out=acc3[:, 1, i:i + 1])
                          else:
                              nc.vector.bn_stats(
                                  stats[m][:, i, :],
                                  xs[m][:, 512 * i:512 * (i + 1)])
                  if m == 3:
                      continue
                  nc.vector.bn_aggr(mv[m][:], stats[m][:])
                  # rhs_m = [mean, var + mean^2]
                  nc.vector.tensor_copy(rhs_m[m][:, 0:1], mv[m][:, 0:1])
                  nc.vector.tensor_tensor(rhs_m[m][:, 1:2], mv[m][:, 0:1],
                                          mv[m][:, 0:1], op=OP.mult)
                  nc.vector.tensor_tensor(rhs_m[m][:, 1:2], rhs_m[m][:, 1:2],
                                          mv[m][:, 1:2], op=OP.add)
              # m=3: rhs = [sum of mean-partials, sum of E[x^2]-partials]
              nc.vector.tensor_reduce(rhs_m[3][:, 0:1], acc3[:, 0, :],
                                      axis=AX.X, op=OP.add)
              nc.vector.tensor_reduce(rhs_m[3][:, 1:2], acc3[:, 1, :],
                                      axis=AX.X, op=OP.add)
              # group sums over channels: [32, 2] = sum_m giT[m].T @ rhs_m
              st32 = accp.tile([NG, 2], f32, tag="aps")
              for m in range(KT):
                  nc.tensor.matmul(st32[:], giT_sb[m][:], rhs_m[m][:],
                                   start=(m == 0), stop=(m == KT - 1))
              gstat = xpool.tile([NG, 2], f32, tag="gstat")
              nc.vector.tensor_scalar_mul(gstat[:], st32[:], 1.0 / 16.0)
              gvar = xpool.tile([NG, 1], f32, tag="gvar")
              nc.vector.tensor_tensor(gvar[:], gstat[:, 0:1], gstat[:, 0:1],
                                      op=OP.mult)
              nc.vector.tensor_tensor(gvar[:], gstat[:, 1:2], gvar[:],
                                      op=OP.subtract)
              # rstd = exp(-0.5 * ln(var + eps))  (Rsqrt ACT is banned)
              eps_sb = xpool.tile([NG, 1], f32, tag="eps")
              nc.vector.memset(eps_sb[:], EPS)
              glog = xpool.tile([NG, 1], f32, tag="glog")
              nc.scalar.activation(glog[:], gvar[:], AF.Ln, bias=eps_sb[:])
              rstd = xpool.tile([NG, 1], f32, tag="rstd")
              nc.scalar.activation(rstd[:], glog[:], AF.Exp, scale=-0.5)

              s_part = [xpool.tile([128, 1], f32, tag=f"sp{m}", name=f"sp{m}")
                        for m in range(KT)]
              t_part = [xpool.tile([128, 1], f32, tag=f"tp{m}", name=f"tp{m}")
                        for m in range(KT)]
              ttmp = [xpool.tile([128, 1], f32, tag=f"tt{m}", name=f"tt{m}")
                      for m in range(KT)]
              mcs = [xpool.tile([128, 1], f32, tag=f"mc{m}", name=f"mc{m}")
                     for m in range(KT)]
              for m in range(KT):
                  mc = accp.tile([128, 1], f32, tag="aps")
                  nc.tensor.matmul(mc[:], gi_sb[:, 128 * m:128 * (m + 1)],
                                   gstat[:, 0:1])
                  nc.vector.tensor_copy(mcs[m][:], mc[:])
              for m in range(KT):
                  rc = accp.tile([128, 1], f32, tag="aps")
                  nc.tensor.matmul(rc[:], gi_sb[:, 128 * m:128 * (m + 1)],
                                   rstd[:])
                  nc.vector.tensor_tensor(s_part[m][:], w_part[m][:], rc[:],
                                          op=OP.mult)
                  nc.vector.tensor_tensor(ttmp[m][:], mcs[m][:], s_part[m][:],
                                          op=OP.mult)
                  nc.vector.tensor_tensor(t_part[m][:], b_part[m][:],
                                          ttmp[m][:], op=OP.subtract)

              ebias_sb = xpool.tile([128, 1], f32, tag="ebias")
              if FP8AV:
                  nc.vector.memset(ebias_sb[:], EXP_BIAS)
              if rep == 0:
                  warm = xpool.tile([1, 2], f32, tag="warm")
                  nc.vector.memset(warm[:], 1.0)
                  wo = xpool.tile([1, 2], f32, tag="warmo")
                  nc.scalar.activation(wo[:], warm[:], AF.Ln)
                  nc.scalar.activation(wo[:], warm[:], AF.Exp)

              # h = x * s + t, emitted per 512-column chunk on demand
              applied = set()

              def apply_chunk(nn):
                  if nn in applied:
                      return
                  applied.add(nn)
                  sl = slice(512 * nn, 512 * (nn + 1))
                  for m in range(KT):
                      nc.vector.tensor_scalar(xs[m][:, sl], xs[m][:, sl],
                                              s_part[m][:], t_part[m][:],
                                              op0=OP.mult, op1=OP.add)

              # producer psum comes from the "st" tag: it borrows rotation
              # turns from the S^T pipeline instead of contending with the
              # long-lived AV accumulators in accp.
              def emit_k_chunk(j, nn):
                  apply_chunk(nn)
                  ps = prp.tile([128, 512], f32, tag="pr")
                  for m in range(KT):
                      nc.tensor.matmul(
                          ps[:], wk_all[:, m, 128 * j:128 * (j + 1)],
                          xs[m][:, 512 * nn:512 * (nn + 1)],
                          start=(m == 0), stop=(m == KT - 1))
                  nc.vector.tensor_scalar_add(
                      k_sb[:, j, 512 * nn:512 * (nn + 1)], ps[:],
                      bk_sb[:, j:j + 1])

              def emit_q_chunk(j, c):
                  # produce q in 512-col chunks (c in {0,1}); own block ==
                  # local columns [0, QBLK) thanks to the host-side rotation
                  apply_chunk(c)
                  ps = prp.tile([128, 512], f32, tag="pr")
                  for m in range(KT):
                      nc.tensor.matmul(
                          ps[:], wq_all[:, m, 128 * j:128 * (j + 1)],
                          xs[m][:, 512 * c:512 * (c + 1)],
                          start=(m == 0), stop=(m == KT - 1))
                  nc.vector.tensor_scalar_add(
                      q_sb[:, j, 512 * c:512 * (c + 1)], ps[:],
                      bq_sb[:, j:j + 1])

              def emit_v_block(j, s):
                  # vT for 128-key block s of pair j: stationary h tile,
                  # moving wv pair slice; v bias rides in as a rank-1
                  # ones_row (x) bv_row matmul into the same psum group.
                  apply_chunk(s // 4)
                  tp = prp.tile([128, 2, 64], f32, tag="pr")
                  for m in range(KT):
                      nc.tensor.matmul(
                          tp[:], xs[m][:, 128 * s:128 * (s + 1)],
                          wv_all[:, m, 128 * j:128 * (j + 1)],
                          start=(m == 0), stop=False)
                  nc.tensor.matmul(tp[:], onesr_sb[:],
                                   bvr_sb[:, 128 * j:128 * (j + 1)],
                                   start=False, stop=True)
                  # single strided copy fills both heads' 64 data columns,
                  # skipping the ones columns
                  nc.vector.tensor_copy(vT_sb[:, j, s, :, 0:64], tp[:])

              # ---- producers + progress tracking --------------------------
              kd = {j: 0 for j in range(NPAIR)}
              qd = set()
              vd = {j: 0 for j in range(NPAIR)}

              def P_k(j, nn):
                  def go():
                      emit_k_chunk(j, nn)
                      kd[j] = max(kd[j], nn + 1)
                  return go

              def P_q(j, c):
                  def go():
                      emit_q_chunk(j, c)
                      qd.add((j, c))
                  return go

              def P_v(j, s):
                  def go():
                      emit_v_block(j, s)
                      vd[j] = max(vd[j], s + 1)
                  return go

              def P_xres(m):
                  def go():
                      nc.sync.dma_start(xres_t[m][:], xres_ext[
                          128 * m:128 * (m + 1), :])
                  return go

              producers = []
              for j in range(NPAIR):
                  if j > 0:
                      producers.append(P_q(j, 0))
                      producers.append(P_k(j, 0))
                  for nn in range(1, NCH):
                      producers.append(P_k(j, nn))
                  producers.append(P_q(j, 1))
                  for s in range(NS):
                      producers.append(P_v(j, s))
              for m in range(KT):
                  producers.append(P_xres(m))
              pidx = [0]

              def run_producer():
                  if pidx[0] < len(producers):
                      producers[pidx[0]]()
                      pidx[0] += 1
                      return True
                  return False

              def ensure(cond_fn):
                  while not cond_fn():
                      if not run_producer():
                          raise RuntimeError("producer underflow")

              # ---- attention ----------------------------------------------
              # flat order within a vtb is head-major: f//NS = head,
              # f%NS = s-tile, so consecutive s pair up for DoubleRow.
              vtbs = [(tb, j) for j in range(NPAIR) for tb in range(NTB)]
              NFLAT = 2 * NS
              groups = [list(range(i, min(i + GSZ, NFLAT)))
                        for i in range(0, NFLAT, GSZ)]
              flat = [(vtb, grp) for vtb in vtbs for grp in groups]

              a_ps_cur = {}
              fin_count = {tb: 0 for tb in range(NTB)}

              def emit_proj_chunk(nn):
                  # proj + bias + residual for own-block columns
                  # [512nn, 512nn+512); po borrows "st" rotation slots
                  sl = slice(TBLK * nn, TBLK * (nn + 1))
                  for mo in range(KT):
                      po = prp.tile([128, TBLK], f32, tag="pr")
                      for mi in range(KT):
                          nc.tensor.matmul(
                              po[:],
                              pw_all[:, mi, 128 * mo:128 * (mo + 1)],
                              a_sb[mi][:, sl],
                              start=(mi == 0), stop=(mi == KT - 1))
                      osb = attnp.tile([128, TBLK], f32, tag="osb", bufs=4)
                      nc.vector.scalar_tensor_tensor(
                          osb[:], po[:], pb_sb[:, mo:mo + 1],
                          xres_t[mo][:, sl],
                          op0=OP.add, op1=OP.add)
                      nc.sync.dma_start(
                          out_ext[128 * mo:128 * (mo + 1), sl], osb[:])

              def emit_st(vtb, grp):
                  tb, j = vtb
                  t0 = TBLK * tb
                  stg = stp.tile([128, GSZ, TBLK], f32, tag="st")
                  for idx, f in enumerate(grp):
                      s, hd = f % NS, f // NS
                      nc.tensor.matmul(
                          stg[:, idx, :],
                          k_sb[64 * hd:64 * (hd + 1), j,
                               128 * s:128 * (s + 1)],
                          q_sb[64 * hd:64 * (hd + 1), j, t0:t0 + TBLK])
                  eg = expp.tile([128, GSZ, TBLK], avdt, tag="eg")
                  if FP8AV:
                      nc.scalar.activation(eg[:, 0:len(grp), :],
                                           stg[:, 0:len(grp), :], AF.Exp,
                                           bias=ebias_sb[:])
                  else:
                      nc.scalar.activation(eg[:, 0:len(grp), :],
                                           stg[:, 0:len(grp), :], AF.Exp)
                  return eg

              def emit_av(vtb, grp, eg):
                  tb, j = vtb
                  if vtb not in a_ps_cur:
                      a_ps_cur[vtb] = accp.tile([65, 2, TBLK], f32,
                                                tag="aps", name="avac")
                  pk = a_ps_cur[vtb]
                  accs = [pk[:, 0, :], pk[:, 1, :]]
                  i = 0
                  while i < len(grp):
                      f = grp[i]
                      s, hd = f % NS, f // NS
                      pair = (FP8AV and i + 1 < len(grp)
                              and grp[i + 1] == f + 1 and s + 1 < NS)
                      if pair:
                          nc.tensor.matmul(
                              accs[hd][:],
                              vT_sb[:, j, s:s + 2, hd, 0:65],
                              eg[:, i:i + 2, :],
                              start=(s == 0), stop=(s + 1 == NS - 1),
                              perf_mode=DR, skip_group_check=True)
                          last_s = s + 1
                          i += 2
                      else:
                          nc.tensor.matmul(
                              accs[hd][:],
                              vT_sb[:, j, s, hd, 0:65],
                              eg[:, i, :],
                              start=(s == 0), stop=(s == NS - 1),
                              skip_group_check=True)
                          last_s = s
                          i += 1
                      if last_s == NS - 1:
                          emit_finalize_head(vtb, hd, accs[hd])

              def emit_finalize_head(vtb, hd, acc):
                  # normalize:  a[c,t] / denom[t];  denom sits in row 64.
                  # Copy out of the aps PSUM slot FIRST so the next vtb's
                  # AV accumulator is not blocked behind the slow 1-lane
                  # reciprocal chain.
                  tb, j = vtb
                  dn = attnp.tile([128, TBLK], f32, tag="rf", bufs=4)
                  nc.vector.tensor_copy(dn[64:65, :], acc[64:65, :])
                  un = attnp.tile([64, TBLK], f32, tag="un", bufs=2)
                  nc.vector.tensor_copy(un[:], acc[0:64, :])
                  rf = attnp.tile([128, TBLK], f32, tag="rf", bufs=4)
                  with nc.allow_low_precision(
                          reason="f32r rounding of softmax recip ~1e-5"):
                      nc.vector.reciprocal(R(rf[64:65, :]), dn[64:65, :])
                  # broadcast 1/denom across partitions via PE ones-column
                  rb = prp.tile([64, TBLK], f32, tag="pr")
                  nc.tensor.matmul(rb[:], R(ones_sb[64:65, 0:64]),
                                   R(rf[64:65, :]))
                  # a lands directly in its proj layout: head 2j+hd ->
                  # a_sb[j], rows 64*hd, own-block cols of tb
                  cols = slice(TBLK * tb, TBLK * (tb + 1))
                  nc.vector.tensor_tensor(
                      a_sb[j][64 * hd:64 * (hd + 1), cols], un[:], rb[:],
                      op=OP.mult)
                  fin_count[tb] += 1
                  if fin_count[tb] == 2 * NPAIR:
                      emit_proj_chunk(tb)

              # bootstrap: k[0] chunk 0 + q[0] tb0, then S^T groups start
              emit_k_chunk(0, 0)
              kd[0] = 1
              emit_q_chunk(0, 0)
              qd.add((0, 0))

              from collections import deque
              backlog = deque()
              nxt = [0]

              def st_ready(vtb, grp):
                  tb, j = vtb
                  need_k = max(f % NS for f in grp) // 4 + 1
                  qc = (TBLK * tb) // 512
                  ensure(lambda: kd[j] >= min(need_k, NCH)
                         and (j, qc) in qd)

              def av_ready(vtb, grp):
                  tb, j = vtb
                  need_v = max(f % NS for f in grp) + 1
                  ensure(lambda: vd[j] >= need_v)

              def sprinkle(n):
                  for _ in range(n):
                      if nxt[0] >= len(flat):
                          return
                      vtb, grp = flat[nxt[0]]
                      st_ready(vtb, grp)
                      backlog.append((vtb, grp, emit_st(vtb, grp)))
                      nxt[0] += 1

              def drain_one():
                  pvtb, pgrp, peg = backlog.popleft()
                  av_ready(pvtb, pgrp)
                  emit_av(pvtb, pgrp, peg)

              # fill the pipeline: PRE groups of S^T/exp before first AV,
              # threading producers between groups to keep ACT fed
              for _ in range(PRE):
                  sprinkle(1)
                  for _ in range(3):
                      run_producer()
              while nxt[0] < len(flat):
                  sprinkle(1)
                  drain_one()
                  run_producer()
              while backlog:
                  drain_one()
              while run_producer():
                  pass

        for rep in range(REPS if reps is None else reps):
            emit_body(rep)

    nc.compile()
    return nc


def prep_in_maps(inputs, L=HW_L):
    x = np.asarray(inputs["x"], dtype=np.float32).reshape(B, C, L)
    qkv_w = np.asarray(inputs["qkv_w"], dtype=np.float32)
    qkv_b = np.asarray(inputs["qkv_b"], dtype=np.float32)
    proj_w = np.asarray(inputs["proj_w"], dtype=np.float32)
    proj_b = np.asarray(inputs["proj_b"], dtype=np.float32)
    norm_w = np.asarray(inputs["norm_w"], dtype=np.float32)
    norm_b = np.asarray(inputs["norm_b"], dtype=np.float32)

    gind = np.zeros((NG, C), dtype=np.float32)
    gind[np.arange(C) // 16, np.arange(C)] = 1.0

    def rows(h, kind):
        s = 192 * h + 64 * kind
        return slice(s, s + 64)

    wq = np.concatenate([qkv_w[rows(h, 0)] for h in range(NH)], 0)
    wk = np.concatenate([qkv_w[rows(h, 1)] for h in range(NH)], 0)
    wv = np.concatenate([qkv_w[rows(h, 2)] for h in range(NH)], 0)
    bq = np.concatenate([qkv_b[rows(h, 0)] for h in range(NH)])
    bk = np.concatenate([qkv_b[rows(h, 1)] for h in range(NH)])
    bv = np.concatenate([qkv_b[rows(h, 2)] for h in range(NH)])

    ones8_dt = ml_dtypes.float8_e4m3 if FP8AV else ml_dtypes.bfloat16
    common = {
        "wqT": np.ascontiguousarray(
            (SCALE * wq).T).astype(ml_dtypes.bfloat16),
        "wkT": np.ascontiguousarray(
            (SCALE * wk).T).astype(ml_dtypes.bfloat16),
        "wvT": np.ascontiguousarray(wv.T).astype(ml_dtypes.bfloat16),
        "pwT": np.ascontiguousarray(proj_w.T).astype(ml_dtypes.bfloat16),
        "bq": np.ascontiguousarray((SCALE * bq).reshape(NPAIR, 128).T),
        "bk": np.ascontiguousarray((SCALE * bk).reshape(NPAIR, 128).T),
        "pb": np.ascontiguousarray(proj_b.reshape(KT, 128).T),
        "bvr": np.ascontiguousarray(
            bv.reshape(1, C)).astype(ml_dtypes.bfloat16),
        "nw": np.ascontiguousarray(norm_w.reshape(C, 1)),
        "nb": np.ascontiguousarray(norm_b.reshape(C, 1)),
        "gind": gind,
        "gindT": np.ascontiguousarray(gind.T),
        "ones8": np.ones((128, 64), dtype=ones8_dt),
        "onesf": np.ones((128, 64), dtype=np.float32),
        "onesr": np.ones((1, 128), dtype=ml_dtypes.bfloat16),
    }
    in_maps = []
    for core in range(N_CORES):
        b, qb = core // 4, core % 4
        m = dict(common)
        # rotate L so this core's query block sits at local cols 0..QBLK-1;
        # attention + group-norm are permutation-invariant over keys
        m["x"] = np.ascontiguousarray(
            np.roll(x[b], -QBLK * qb, axis=1)).astype(ml_dtypes.bfloat16)
        m["xres"] = np.ascontiguousarray(
            x[b, :, QBLK * qb:QBLK * (qb + 1)])
        in_maps.append(m)
    return in_maps


def gather_output(results, L=HW_L):
    out = np.empty((B, C, L), dtype=np.float32)
    for core in range(N_CORES):
        b, qb = core // 4, core % 4
        out[b, :, QBLK * qb:QBLK * (qb + 1)] = results[core]["out"]
    s = int(np.sqrt(L))
    return out.reshape(B, C, s, s)


_NC_CACHE = {}


def get_nc(L=HW_L, reps=None):
    key = (L, reps)
    if key not in _NC_CACHE:
        _NC_CACHE[key] = build_nc(L, reps=reps)
    return _NC_CACHE[key]


def kernel(**inputs):
    from concourse.bass_utils import run_bass_kernel_spmd

    nc = get_nc()
    in_maps = prep_in_maps(inputs)
    res = run_bass_kernel_spmd(nc, in_maps, core_ids=list(range(N_CORES)))
    return gather_output(res.results)


if __name__ == "__main__":
    rng = np.random.default_rng(0)
    inputs = {
        "x": rng.standard_normal((B, C, 64, 64), dtype=np.float32),
        "norm_w": rng.standard_normal(C, dtype=np.float32) * 0.1 + 1.0,
        "norm_b": rng.standard_normal(C, dtype=np.float32) * 0.1,
        "qkv_w": (rng.standard_normal((3 * C, C), dtype=np.float32)
                  / np.sqrt(C)),
        "qkv_b": rng.standard_normal(3 * C, dtype=np.float32) * 0.02,
        "proj_w": (rng.standard_normal((C, C), dtype=np.float32)
                   / np.sqrt(C)),
        "proj_b": rng.standard_normal(C, dtype=np.float32) * 0.02,
    }
    out = kernel(**inputs)
    print("kernel output", out.shape, out.dtype, float(np.abs(out).mean()))


# revision 25
# speedup vs baseline: 1.0810x; 1.0810x over previous
"""Trainium2 Bass kernel for nn_AttentionBlock (B=2, C=512, L=64x64, 8 heads).

Sharding v2: 8 cores = 2 (batch) x 4 (query-blocks of 1024 columns).
Each core computes group-norm + k,v for ALL 8 heads (redundant across the
4 cores of a batch), q/attention/proj only for its own 1024 query columns,
and writes a [512, 1024] fp32 output slice.  NO collectives -- the host
concatenates the L-slices.  This trades ~40us of redundant k/v PE work
for the old AllGather + inter-core sync (~300us).

The host ROTATES x along L per core (np.roll by -1024*qb) so each core's
query block sits at local columns 0-1023; attention and group-norm are
permutation-invariant over keys/columns, so this is exact.

Heads are processed as 4 "pairs" (2 heads per 128 partitions); virtual
t-blocks are (tb, pair) in PAIR-major order, tb over QBLK/TBLK=4 blocks
of 256 queries, so the k/v/q producer stream for pair j+1 spreads
evenly under pair j's attention.  Within a vtb the flat order is
head-major (head A s0..s31, then head B) so consecutive s-tiles pair up
for fp8 DoubleRow AV matmuls (2 s-tiles per pass).  exp is emitted with
input bias -2 so softmax weights fit fp8e4m3's 240 max (uniform scaling
cancels in the softmax).  Each head finalizes as soon as its s=31 AV
lands; when the last head of a t-block column finishes, that column's
proj + bias + residual drains inline.

PSUM budget (8 banks): S^T/exp groups 2 x [128,GSZ=6,256] (stp "st",
6 banks, exclusively for the S^T pipeline), the packed per-vtb AV
accumulator [65,2,256] (accp "aps", 1 bank, both heads side by side),
and 1 bank (prp "pr") for producer/proj/recip-broadcast matmuls so
they never steal S^T slot rotation turns.

DMA routing: x (8 half-row transfers) + xres + out on the SP HWDGE
queue; weights / biases / constants via gpsimd software DGE on the
otherwise-idle Pool engine, so neither the ACT sequencer nor the
shared HWDGE delays the group-norm stats or the first exp.  m=3's
channel stats run on ACT (activation accum_out: Copy(x/4096) sums the
mean, Square(x/64) sums E[x^2]) while DVE bn_stats covers m=0..2.

Matmul dtypes: bf16 S^T / qkv / proj, fp8e4m3 AV (DoubleRow where s-tiles
pair), fp32 group-norm matmuls, float32r 1/denom broadcast.
"""

import sys

if "/opt/trn_rl_repo" not in sys.path:
    sys.path.insert(0, "/opt/trn_rl_repo")

import numpy as np
import ml_dtypes

B, C = 2, 512
HW_L = 4096          # 64*64
NH, CHD, NG = 8, 64, 32
EPS = 1e-5
N_CORES = 8
SCALE = 1.0 / np.sqrt(np.sqrt(CHD))
QBLK = 1024          # query columns per core
TBLK = 256           # t-block (query block) size
NTB = QBLK // TBLK   # t-blocks per pair (4)
NPAIR = 4            # head pairs
KT = C // 128        # 4 input-channel tiles
GSZ = 6              # exp group size (s-tiles per [128,GSZ,TBLK] psum tile)
EGB = 18             # exp-tile pool bufs
PRE = 12             # S^T/exp groups emitted ahead of their AV
FP8AV = True         # fp8e4m3 AV with DoubleRow pairing
EXP_BIAS = -2.0      # exp(s-2): keeps weights under fp8e4m3 max (240)
REPS = 1             # emit the whole body N times (differential timing)


def build_nc(L=HW_L, reps=None):
    import concourse.bass as bass
    import concourse.tile as tile
    from concourse import bacc, mybir
    from contextlib import ExitStack

    f32 = mybir.dt.float32
    f32r = mybir.dt.float32r
    bf16 = mybir.dt.bfloat16
    f8 = mybir.dt.float8e4
    avdt = f8 if FP8AV else bf16
    AF = mybir.ActivationFunctionType
    OP = mybir.AluOpType
    AX = mybir.AxisListType
    DR = mybir.MatmulPerfMode.DoubleRow

    def R(ap):
        # reinterpret fp32 operand as float32r: 1 PE cycle/row (vs 4 for
        # fp32) when the output free size is >= 256
        return ap.bitcast(f32r)

    NS = L // 128           # 32 key s-tiles per pair
    NCH = L // 512          # 8 512-wide chunks of L
    VPAD = 80 if FP8AV else 65   # vT inner stride (DoubleRow needs %16==0)

    nc = bacc.Bacc("TRN2", target_bir_lowering=False, debug=False,
                   num_devices=N_CORES)

    x_ext = nc.dram_tensor("x", [C, L], bf16, kind="ExternalInput")
    xres_ext = nc.dram_tensor("xres", [C, QBLK], f32, kind="ExternalInput")
    wq_ext = nc.dram_tensor("wqT", [C, C], bf16, kind="ExternalInput")
    wk_ext = nc.dram_tensor("wkT", [C, C], bf16, kind="ExternalInput")
    wv_ext = nc.dram_tensor("wvT", [C, C], bf16, kind="ExternalInput")
    pw_ext = nc.dram_tensor("pwT", [C, C], bf16, kind="ExternalInput")
    bq_ext = nc.dram_tensor("bq", [128, NPAIR], f32, kind="ExternalInput")
    bk_ext = nc.dram_tensor("bk", [128, NPAIR], f32, kind="ExternalInput")
    pb_ext = nc.dram_tensor("pb", [128, KT], f32, kind="ExternalInput")
    bvr_ext = nc.dram_tensor("bvr", [1, C], bf16, kind="ExternalInput")
    nw_ext = nc.dram_tensor("nw", [C, 1], f32, kind="ExternalInput")
    nb_ext = nc.dram_tensor("nb", [C, 1], f32, kind="ExternalInput")
    gi_ext = nc.dram_tensor("gind", [NG, C], f32, kind="ExternalInput")
    giT_ext = nc.dram_tensor("gindT", [C, NG], f32, kind="ExternalInput")
    ones8_ext = nc.dram_tensor("ones8", [128, 64], avdt,
                               kind="ExternalInput")
    onesf_ext = nc.dram_tensor("onesf", [128, 64], f32, kind="ExternalInput")
    onesr_ext = nc.dram_tensor("onesr", [1, 128], bf16, kind="ExternalInput")
    out_ext = nc.dram_tensor("out", [C, QBLK], f32, kind="ExternalOutput")

    with tile.TileContext(nc, num_cores=N_CORES) as tc, ExitStack() as ctx:
        pers = ctx.enter_context(tc.tile_pool(name="pers", bufs=1))
        # accp: 1 bank -- the packed per-vtb AV accumulator [65, 2, TBLK]
        # (both heads side by side) plus the small group-norm matmuls
        accp = ctx.enter_context(
            tc.tile_pool(name="accp", bufs=1, space="PSUM"))

        # ---- persistent tiles (loads deferred until after the x DMAs so
        # the group-norm stats never wait behind weight descriptors on the
        # shared HWDGE) ----------------------------------------------------
        # weight layout: w*_all[:, m, :] holds rows 128m..128m+127 of the
        # [C, C] transposed weight; stationary AP for (m, pair j) is
        # w*_all[:, m, 128j:128j+128].
        wq_all = pers.tile([128, KT, C], bf16, tag="wq")
        wk_all = pers.tile([128, KT, C], bf16, tag="wk")
        wv_all = pers.tile([128, KT, C], bf16, tag="wv")
        pw_all = pers.tile([128, KT, C], bf16, tag="pw")
        bq_sb = pers.tile([128, NPAIR], f32, tag="bq")
        bk_sb = pers.tile([128, NPAIR], f32, tag="bk")
        pb_sb = pers.tile([128, KT], f32, tag="pb")
        w_part = [pers.tile([128, 1], f32, tag=f"nw{m}", name=f"nw{m}")
                  for m in range(KT)]
        b_part = [pers.tile([128, 1], f32, tag=f"nb{m}", name=f"nb{m}")
                  for m in range(KT)]
        gi_sb = pers.tile([NG, C], f32, tag="gi")
        giT_sb = [pers.tile([128, NG], f32, tag=f"giT{m}", name=f"giT{m}")
                  for m in range(KT)]
        ones_sb = pers.tile([128, 64], f32, tag="ones")
        onesr_sb = pers.tile([1, 128], bf16, tag="onesr")
        bvr_sb = pers.tile([1, C], bf16, tag="bvr")

        def emit_pers_loads():
            # ordered by first use: group-norm stats path, then k/q weights
            # for the bootstrap, then the rest
            nc.gpsimd.dma_start(gi_sb[:], gi_ext[:])
            for m in range(KT):
                nc.gpsimd.dma_start(giT_sb[m][:],
                                    giT_ext[128 * m:128 * (m + 1), :])
                nc.gpsimd.dma_start(w_part[m][:],
                                    nw_ext[128 * m:128 * (m + 1), :])
                nc.gpsimd.dma_start(b_part[m][:],
                                    nb_ext[128 * m:128 * (m + 1), :])
            for m in range(KT):
                nc.gpsimd.dma_start(wk_all[:, m, :],
                                    wk_ext[128 * m:128 * (m + 1), :])
            nc.gpsimd.dma_start(bk_sb[:], bk_ext[:])
            for m in range(KT):
                nc.gpsimd.dma_start(wq_all[:, m, :],
                                    wq_ext[128 * m:128 * (m + 1), :])
            nc.gpsimd.dma_start(bq_sb[:], bq_ext[:])
            nc.gpsimd.dma_start(R(ones_sb[:]), R(onesf_ext[:]))
            nc.gpsimd.dma_start(onesr_sb[:], onesr_ext[:])
            nc.gpsimd.dma_start(bvr_sb[:], bvr_ext[:])
            for m in range(KT):
                nc.gpsimd.dma_start(wv_all[:, m, :],
                                    wv_ext[128 * m:128 * (m + 1), :])
            for j in range(NPAIR):
                nc.gpsimd.dma_start(vT_sb[:, j, :, :, 64:65],
                                    ones8_ext[:, 0:2 * NS])
            for m in range(KT):
                nc.gpsimd.dma_start(pw_all[:, m, :],
                                    pw_ext[128 * m:128 * (m + 1), :])
            nc.gpsimd.dma_start(pb_sb[:], pb_ext[:])

        # persistent activation tensors
        q_sb = pers.tile([128, NPAIR, QBLK], bf16, tag="q")
        k_sb = pers.tile([128, NPAIR, L], bf16, tag="k")
        # vT laid out [keys, pair, s-tile, head, VPAD] -- col 64 of each
        # head is the all-ones column accumulating the softmax denominator
        vT_sb = pers.tile([128, NPAIR, NS, 2, VPAD], avdt, tag="vT")
        a_sb = [pers.tile([128, QBLK], bf16, tag=f"a{m}", name=f"a{m}")
                for m in range(KT)]

        def emit_body(rep):
          with (
            tc.tile_pool(name=f"xpool{rep}", bufs=1) as xpool,
            tc.tile_pool(name=f"attn{rep}", bufs=1) as attnp,
          ):
            xres_t = [xpool.tile([128, QBLK], f32, tag=f"xr{m}",
                                 name=f"xr{m}") for m in range(KT)]
            with (
              tc.tile_pool(name=f"stp{rep}",
                           bufs=(8 - TBLK // 256
                                 - (2 if TBLK == 512 else 1))
                           // ((GSZ * TBLK * 4) // 2048),
                           space="PSUM") as stp,
              tc.tile_pool(name=f"prp{rep}",
                           bufs=(2 if TBLK == 512 else 1),
                           space="PSUM") as prp,
              tc.tile_pool(name=f"expp{rep}", bufs=EGB) as expp,
            ):
              xs = [xpool.tile([128, L], bf16, tag=f"x{m}", name=f"x{m}")
                    for m in range(KT)]
              stats = [xpool.tile([128, L // 512, 6], f32, tag=f"bs{m}",
                                  name=f"bs{m}") for m in range(KT)]
              mv = [xpool.tile([128, 2], f32, tag=f"mv{m}", name=f"mv{m}")
                    for m in range(KT)]
              rhs_m = [xpool.tile([128, 2], f32, tag=f"rh{m}", name=f"rh{m}")
                       for m in range(KT)]
              # x in 8 half-row DMAs (fewer descriptors on the shared
              # HWDGE).  m=3 loads first and its channel stats run on the
              # (otherwise idle) ACT engine via activation accum_out:
              # Copy(x/4096) accumulates the mean, Square(x/64) accumulates
              # E[x^2] (= sum(x^2)/4096).  m=0..2 use DVE bn_stats per
              # 512-chunk as each half lands.
              HHALF = L // 2
              acc3 = xpool.tile([128, 2, L // 512], f32, tag="acc3")
              ascr = [xpool.tile([128, 512], bf16, tag=f"ascr{z}",
                                 name=f"ascr{z}") for z in range(2)]
              for m in [3, 0, 1, 2]:
                  for hh in range(2):
                      nc.sync.dma_start(
                          xs[m][:, HHALF * hh:HHALF * (hh + 1)],
                          x_ext[128 * m:128 * (m + 1),
                                HHALF * hh:HHALF * (hh + 1)])
                      if m == 3 and hh == 0 and rep == 0:
                          emit_pers_loads()
                      for i in range(4 * hh, 4 * (hh + 1)):
                          if m == 3:
                              sl = xs[m][:, 512 * i:512 * (i + 1)]
                              nc.scalar.activation(
                                  ascr[0][:], sl, AF.Copy,
                                  scale=1.0 / 4096.0,
                                  accum_out=acc3[:, 0, i:i + 1])
                              nc.scalar.activation(
                                  ascr[1][:], sl, AF.Square,
                                  scale=1.0 / 64.0,
                                  accum_out=acc3[:, 1, i:i + 1])
                          else:
                              nc.vector.bn_stats(
                                  stats[m][:, i, :],
                                  xs[m][:, 512 * i:512 * (i + 1)])
                  if m == 3:
                      continue
                  nc.vector.bn_aggr(mv[m][:], stats[m][:])
                  # rhs_m = [mean, var + mean^2]
                  nc.vector.tensor_copy(rhs_m[m][:, 0:1], mv[m][:, 0:1])
                  nc.vector.tensor_tensor(rhs_m[m][:, 1:2], mv[m][:, 0:1],
                                          mv[m][:, 0:1], op=OP.mult)
                  nc.vector.tensor_tensor(rhs_m[m][:, 1:2], rhs_m[m][:, 1:2],
                                          mv[m][:, 1:2], op=OP.add)
              # m=3: rhs = [sum of mean-partials, sum of E[x^2]-partials]
              nc.vector.tensor_reduce(rhs_m[3][:, 0:1], acc3[:, 0, :],
                                      axis=AX.X, op=OP.add)
              nc.vector.tensor_reduce(rhs_m[3][:, 1:2], acc3[:, 1, :],
                                      axis=AX.X, op=OP.add)
              # group sums over channels: [32, 2] = sum_m giT[m].T @ rhs_m
              st32 = accp.tile([NG, 2], f32, tag="aps")
              for m in range(KT):
                  nc.tensor.matmul(st32[:], giT_sb[m][:], rhs_m[m][:],
                                   start=(m == 0), stop=(m == KT - 1))
              gstat = xpool.tile([NG, 2], f32, tag="gstat")
              nc.vector.tensor_scalar_mul(gstat[:], st32[:], 1.0 / 16.0)
              gvar = xpool.tile([NG, 1], f32, tag="gvar")
              nc.vector.tensor_tensor(gvar[:], gstat[:, 0:1], gstat[:, 0:1],
                                      op=OP.mult)
              nc.vector.tensor_tensor(gvar[:], gstat[:, 1:2], gvar[:],
                                      op=OP.subtract)
              # rstd = exp(-0.5 * ln(var + eps))  (Rsqrt ACT is banned)
              eps_sb = xpool.tile([NG, 1], f32, tag="eps")
              nc.vector.memset(eps_sb[:], EPS)
              glog = xpool.tile([NG, 1], f32, tag="glog")
              nc.scalar.activation(glog[:], gvar[:], AF.Ln, bias=eps_sb[:])
              rstd = xpool.tile([NG, 1], f32, tag="rstd")
              nc.scalar.activation(rstd[:], glog[:], AF.Exp, scale=-0.5)

              s_part = [xpool.tile([128, 1], f32, tag=f"sp{m}", name=f"sp{m}")
                        for m in range(KT)]
              t_part = [xpool.tile([128, 1], f32, tag=f"tp{m}", name=f"tp{m}")
                        for m in range(KT)]
              ttmp = [xpool.tile([128, 1], f32, tag=f"tt{m}", name=f"tt{m}")
                      for m in range(KT)]
              mcs = [xpool.tile([128, 1], f32, tag=f"mc{m}", name=f"mc{m}")
                     for m in range(KT)]
              for m in range(KT):
                  mc = accp.tile([128, 1], f32, tag="aps")
                  nc.tensor.matmul(mc[:], gi_sb[:, 128 * m:128 * (m + 1)],
                                   gstat[:, 0:1])
                  nc.vector.tensor_copy(mcs[m][:], mc[:])
              for m in range(KT):
                  rc = accp.tile([128, 1], f32, tag="aps")
                  nc.tensor.matmul(rc[:], gi_sb[:, 128 * m:128 * (m + 1)],
                                   rstd[:])
                  nc.vector.tensor_tensor(s_part[m][:], w_part[m][:], rc[:],
                                          op=OP.mult)
                  nc.vector.tensor_tensor(ttmp[m][:], mcs[m][:], s_part[m][:],
                                          op=OP.mult)
                  nc.vector.tensor_tensor(t_part[m][:], b_part[m][:],
                                          ttmp[m][:], op=OP.subtract)

              ebias_sb = xpool.tile([128, 1], f32, tag="ebias")
              if FP8AV:
                  nc.vector.memset(ebias_sb[:], EXP_BIAS)
              if rep == 0:
                  warm = xpool.tile([1, 2], f32, tag="warm")
                  nc.vector.memset(warm[:], 1.0)
                  wo = xpool.tile([1, 2], f32, tag="warmo")
                  nc.scalar.activation(wo[:], warm[:], AF.Ln)
                  nc.scalar.activation(wo[:], warm[:], AF.Exp)

              # h = x * s + t, emitted per 512-column chunk on demand
              applied = set()

              def apply_chunk(nn):
                  if nn in applied:
                      return
                  applied.add(nn)
                  sl = slice(512 * nn, 512 * (nn + 1))
                  for m in range(KT):
                      nc.vector.tensor_scalar(xs[m][:, sl], xs[m][:, sl],
                                              s_part[m][:], t_part[m][:],
                                              op0=OP.mult, op1=OP.add)

              # producer psum comes from the "st" tag: it borrows rotation
              # turns from the S^T pipeline instead of contending with the
              # long-lived AV accumulators in accp.
              def emit_k_chunk(j, nn):
                  apply_chunk(nn)
                  ps = prp.tile([128, 512], f32, tag="pr")
                  for m in range(KT):
                      nc.tensor.matmul(
                          ps[:], wk_all[:, m, 128 * j:128 * (j + 1)],
                          xs[m][:, 512 * nn:512 * (nn + 1)],
                          start=(m == 0), stop=(m == KT - 1))
                  nc.vector.tensor_scalar_add(
                      k_sb[:, j, 512 * nn:512 * (nn + 1)], ps[:],
                      bk_sb[:, j:j + 1])

              def emit_q_chunk(j, c):
                  # produce q in 512-col chunks (c in {0,1}); own block ==
                  # local columns [0, QBLK) thanks to the host-side rotation
                  apply_chunk(c)
                  ps = prp.tile([128, 512], f32, tag="pr")
                  for m in range(KT):
                      nc.tensor.matmul(
                          ps[:], wq_all[:, m, 128 * j:128 * (j + 1)],
                          xs[m][:, 512 * c:512 * (c + 1)],
                          start=(m == 0), stop=(m == KT - 1))
                  nc.vector.tensor_scalar_add(
                      q_sb[:, j, 512 * c:512 * (c + 1)], ps[:],
                      bq_sb[:, j:j + 1])

              def emit_v_block(j, s):
                  # vT for 128-key block s of pair j: stationary h tile,
                  # moving wv pair slice; v bias rides in as a rank-1
                  # ones_row (x) bv_row matmul into the same psum group.
                  apply_chunk(s // 4)
                  tp = prp.tile([128, 2, 64], f32, tag="pr")
                  for m in range(KT):
                      nc.tensor.matmul(
                          tp[:], xs[m][:, 128 * s:128 * (s + 1)],
                          wv_all[:, m, 128 * j:128 * (j + 1)],
                          start=(m == 0), stop=False)
                  nc.tensor.matmul(tp[:], onesr_sb[:],
                                   bvr_sb[:, 128 * j:128 * (j + 1)],
                                   start=False, stop=True)
                  # single strided copy fills both heads' 64 data columns,
                  # skipping the ones columns
                  nc.vector.tensor_copy(vT_sb[:, j, s, :, 0:64], tp[:])

              # ---- producers + progress tracking --------------------------
              kd = {j: 0 for j in range(NPAIR)}
              qd = set()
              vd = {j: 0 for j in range(NPAIR)}

              def P_k(j, nn):
                  def go():
                      emit_k_chunk(j, nn)
                      kd[j] = max(kd[j], nn + 1)
                  return go

              def P_q(j, c):
                  def go():
                      emit_q_chunk(j, c)
                      qd.add((j, c))
                  return go

              def P_v(j, s):
                  def go():
                      emit_v_block(j, s)
                      vd[j] = max(vd[j], s + 1)
                  return go

              def P_xres(m):
                  def go():
                      nc.sync.dma_start(xres_t[m][:], xres_ext[
                          128 * m:128 * (m + 1), :])
                  return go

              producers = []
              for j in range(NPAIR):
                  if j > 0:
                      producers.append(P_q(j, 0))
                      producers.append(P_k(j, 0))
                  for nn in range(1, NCH):
                      producers.append(P_k(j, nn))
                  producers.append(P_q(j, 1))
                  for s in range(NS):
                      producers.append(P_v(j, s))
              for m in range(KT):
                  producers.append(P_xres(m))
              pidx = [0]

              def run_producer():
                  if pidx[0] < len(producers):
                      producers[pidx[0]]()
                      pidx[0] += 1
                      return True
                  return False

              def ensure(cond_fn):
                  while not cond_fn():
                      if not run_producer():
                          raise RuntimeError("producer underflow")

              # ---- attention ----------------------------------------------
              # flat order within a vtb is head-major: f//NS = head,
              # f%NS = s-tile, so consecutive s pair up for DoubleRow.
              vtbs = [(tb, j) for j in range(NPAIR) for tb in range(NTB)]
              NFLAT = 2 * NS
              groups = [list(range(i, min(i + GSZ, NFLAT)))
                        for i in range(0, NFLAT, GSZ)]
              flat = [(vtb, grp) for vtb in vtbs for grp in groups]

              a_ps_cur = {}
              fin_count = {tb: 0 for tb in range(NTB)}

              def emit_proj_chunk(nn):
                  # proj + bias + residual for own-block columns
                  # [512nn, 512nn+512); po borrows "st" rotation slots
                  sl = slice(TBLK * nn, TBLK * (nn + 1))
                  for mo in range(KT):
                      po = prp.tile([128, TBLK], f32, tag="pr")
                      for mi in range(KT):
                          nc.tensor.matmul(
                              po[:],
                              pw_all[:, mi, 128 * mo:128 * (mo + 1)],
                              a_sb[mi][:, sl],
                              start=(mi == 0), stop=(mi == KT - 1))
                      osb = attnp.tile([128, TBLK], f32, tag="osb", bufs=4)
                      nc.vector.scalar_tensor_tensor(
                          osb[:], po[:], pb_sb[:, mo:mo + 1],
                          xres_t[mo][:, sl],
                          op0=OP.add, op1=OP.add)
                      nc.sync.dma_start(
                          out_ext[128 * mo:128 * (mo + 1), sl], osb[:])

              def emit_st(vtb, grp):
                  tb, j = vtb
                  t0 = TBLK * tb
                  stg = stp.tile([128, GSZ, TBLK], f32, tag="st")
                  for idx, f in enumerate(grp):
                      s, hd = f % NS, f // NS
                      nc.tensor.matmul(
                          stg[:, idx, :],
                          k_sb[64 * hd:64 * (hd + 1), j,
                               128 * s:128 * (s + 1)],
                          q_sb[64 * hd:64 * (hd + 1), j, t0:t0 + TBLK])
                  eg = expp.tile([128, GSZ, TBLK], avdt, tag="eg")
                  if FP8AV:
                      nc.scalar.activation(eg[:, 0:len(grp), :],
                                           stg[:, 0:len(grp), :], AF.Exp,
                                           bias=ebias_sb[:])
                  else:
                      nc.scalar.activation(eg[:, 0:len(grp), :],
                                           stg[:, 0:len(grp), :], AF.Exp)
                  return eg

              def emit_av(vtb, grp, eg):
                  tb, j = vtb
                  if vtb not in a_ps_cur:
                      a_ps_cur[vtb] = accp.tile([65, 2, TBLK], f32,
                                                tag="aps", name="avac")
                  pk = a_ps_cur[vtb]
                  accs = [pk[:, 0, :], pk[:, 1, :]]
                  i = 0
                  while i < len(grp):
                      f = grp[i]
                      s, hd = f % NS, f // NS
                      pair = (FP8AV and i + 1 < len(grp)
                              and grp[i + 1] == f + 1 and s + 1 < NS)
                      if pair:
                          nc.tensor.matmul(
                              accs[hd][:],
                              vT_sb[:, j, s:s + 2, hd, 0:65],
                              eg[:, i:i + 2, :],
                              start=(s == 0), stop=(s + 1 == NS - 1),
                              perf_mode=DR, skip_group_check=True)
                          last_s = s + 1
                          i += 2
                      else:
                          nc.tensor.matmul(
                              accs[hd][:],
                              vT_sb[:, j, s, hd, 0:65],
                              eg[:, i, :],
                              start=(s == 0), stop=(s == NS - 1),
                              skip_group_check=True)
                          last_s = s
                          i += 1
                      if last_s == NS - 1:
                          emit_finalize_head(vtb, hd, accs[hd])

              def emit_finalize_head(vtb, hd, acc):
                  # normalize:  a[c,t] / denom[t];  denom sits in row 64.
                  # Copy out of the aps PSUM slot FIRST so the next vtb's
                  # AV accumulator is not blocked behind the slow 1-lane
                  # reciprocal chain.
                  tb, j = vtb
                  dn = attnp.tile([128, TBLK], f32, tag="rf", bufs=4)
                  nc.vector.tensor_copy(dn[64:65, :], acc[64:65, :])
                  un = attnp.tile([64, TBLK], f32, tag="un", bufs=2)
                  nc.vector.tensor_copy(un[:], acc[0:64, :])
                  rf = attnp.tile([128, TBLK], f32, tag="rf", bufs=4)
                  with nc.allow_low_precision(
                          reason="f32r rounding of softmax recip ~1e-5"):
                      nc.vector.reciprocal(R(rf[64:65, :]), dn[64:65, :])
                  # broadcast 1/denom across partitions via PE ones-column
                  rb = prp.tile([64, TBLK], f32, tag="pr")
                  nc.tensor.matmul(rb[:], R(ones_sb[64:65, 0:64]),
                                   R(rf[64:65, :]))
                  # a lands directly in its proj layout: head 2j+hd ->
                  # a_sb[j], rows 64*hd, own-block cols of tb
                  cols = slice(TBLK * tb, TBLK * (tb + 1))
                  nc.vector.tensor_tensor(
                      a_sb[j][64 * hd:64 * (hd + 1), cols], un[:], rb[:],
                      op=OP.mult)
                  fin_count[tb] += 1
                  if fin_count[tb] == 2 * NPAIR:
                      emit_proj_chunk(tb)

              # bootstrap: k[0] chunk 0 + q[0] tb0, then S^T groups start
              emit_k_chunk(0, 0)
              kd[0] = 1
              emit_q_chunk(0, 0)
              qd.add((0, 0))

              from collections import deque
              backlog = deque()
              nxt = [0]

              def st_ready(vtb, grp):
                  tb, j = vtb
                  need_k = max(f % NS for f in grp) // 4 + 1
                  qc = (TBLK * tb) // 512
                  ensure(lambda: kd[j] >= min(need_k, NCH)
                         and (j, qc) in qd)

              def av_ready(vtb, grp):
                  tb, j = vtb
                  need_v = max(f % NS for f in grp) + 1
                  ensure(lambda: vd[j] >= need_v)

              def sprinkle(n):
                  for _ in range(n):
                      if nxt[0] >= len(flat):
                          return
                      vtb, grp = flat[nxt[0]]
                      st_ready(vtb, grp)
                      backlog.append((vtb, grp, emit_st(vtb, grp)))
                      nxt[0] += 1

              def drain_one():
                  pvtb, pgrp, peg = backlog.popleft()
                  av_ready(pvtb, pgrp)
                  emit_av(pvtb, pgrp, peg)

              # fill the pipeline: PRE groups of S^T/exp before first AV,
              # threading producers between groups to keep ACT fed
              for _ in range(PRE):
                  sprinkle(1)
                  for _ in range(3):
                      run_producer()
              while nxt[0] < len(flat):
                  sprinkle(1)
                  drain_one()
                  run_producer()
              while backlog:
                  drain_one()
              while run_producer():
                  pass

        for rep in range(REPS if reps is None else reps):
            emit_body(rep)

    nc.compile()
    return nc


def prep_in_maps(inputs, L=HW_L):
    x = np.asarray(inputs["x"], dtype=np.float32).reshape(B, C, L)
    qkv_w = np.asarray(inputs["qkv_w"], dtype=np.float32)
    qkv_b = np.asarray(inputs["qkv_b"], dtype=np.float32)
    proj_w = np.asarray(inputs["proj_w"], dtype=np.float32)
    proj_b = np.asarray(inputs["proj_b"], dtype=np.float32)
    norm_w = np.asarray(inputs["norm_w"], dtype=np.float32)
    norm_b = np.asarray(inputs["norm_b"], dtype=np.float32)

    gind = np.zeros((NG, C), dtype=np.float32)
    gind[np.arange(C) // 16, np.arange(C)] = 1.0

    def rows(h, kind):
        s = 192 * h + 64 * kind
        return slice(s, s + 64)

    wq = np.concatenate([qkv_w[rows(h, 0)] for h in range(NH)], 0)
    wk = np.concatenate([qkv_w[rows(h, 1)] for h in range(NH)], 0)
    wv = np.concatenate([qkv_w[rows(h, 2)] for h in range(NH)], 0)
    bq = np.concatenate([qkv_b[rows(h, 0)] for h in range(NH)])
    bk = np.concatenate([qkv_b[rows(h, 1)] for h in range(NH)])
    bv = np.concatenate([qkv_b[rows(h, 2)] for h in range(NH)])

    ones8_dt = ml_dtypes.float8_e4m3 if FP8AV else ml_dtypes.bfloat16
    common = {
        "wqT": np.ascontiguousarray(
            (SCALE * wq).T).astype(ml_dtypes.bfloat16),
        "wkT": np.ascontiguousarray(
            (SCALE * wk).T).astype(ml_dtypes.bfloat16),
        "wvT": np.ascontiguousarray(wv.T).astype(ml_dtypes.bfloat16),
        "pwT": np.ascontiguousarray(proj_w.T).astype(ml_dtypes.bfloat16),
        "bq": np.ascontiguousarray((SCALE * bq).reshape(NPAIR, 128).T),
        "bk": np.ascontiguousarray((SCALE * bk).reshape(NPAIR, 128).T),
        "pb": np.ascontiguousarray(proj_b.reshape(KT, 128).T),
        "bvr": np.ascontiguousarray(
            bv.reshape(1, C)).astype(ml_dtypes.bfloat16),
        "nw": np.ascontiguousarray(norm_w.reshape(C, 1)),
        "nb": np.ascontiguousarray(norm_b.reshape(C, 1)),
        "gind": gind,
        "gindT": np.ascontiguousarray(gind.T),
        "ones8": np.ones((128, 64), dtype=ones8_dt),
        "onesf": np.ones((128, 64), dtype=np.float32),
        "onesr": np.ones((1, 128), dtype=ml_dtypes.bfloat16),
    }
    in_maps = []
    for core in range(N_CORES):
        b, qb = core // 4, core % 4
        m = dict(common)
        # rotate L so this core's query block sits at local cols 0..QBLK-1;
        # attention + group-norm are permutation-invariant over keys
        m["x"] = np.ascontiguousarray(
            np.roll(x[b], -QBLK * qb, axis=1)).astype(ml_dtypes.bfloat16)
        m["xres"] = np.ascontiguousarray(
            x[b, :, QBLK * qb:QBLK * (qb + 1)])
        in_maps.append(m)
    return in_maps


def gather_output(results, L=HW_L):
    out = np.empty((B, C, L), dtype=np.float32)
    for core in range(N_CORES):
        b, qb = core // 4, core % 4
        out[b, :, QBLK * qb:QBLK * (qb + 1)] = results[core]["out"]
    s = int(np.sqrt(L))
    return out.reshape(B, C, s, s)


_NC_CACHE = {}


def get_nc(L=HW_L, reps=None):
    key = (L, reps)
    if key not in _NC_CACHE:
        _NC_CACHE[key] = build_nc(L, reps=reps)
    return _NC_CACHE[key]


def kernel(**inputs):
    from concourse.bass_utils import run_bass_kernel_spmd

    nc = get_nc()
    in_maps = prep_in_maps(inputs)
    res = run_bass_kernel_spmd(nc, in_maps, core_ids=list(range(N_CORES)))
    return gather_output(res.results)


if __name__ == "__main__":
    rng = np.random.default_rng(0)
    inputs = {
        "x": rng.standard_normal((B, C, 64, 64), dtype=np.float32),
        "norm_w": rng.standard_normal(C, dtype=np.float32) * 0.1 + 1.0,
        "norm_b": rng.standard_normal(C, dtype=np.float32) * 0.1,
        "qkv_w": (rng.standard_normal((3 * C, C), dtype=np.float32)
                  / np.sqrt(C)),
        "qkv_b": rng.standard_normal(3 * C, dtype=np.float32) * 0.02,
        "proj_w": (rng.standard_normal((C, C), dtype=np.float32)
                   / np.sqrt(C)),
        "proj_b": rng.standard_normal(C, dtype=np.float32) * 0.02,
    }
    out = kernel(**inputs)
    print("kernel output", out.shape, out.dtype, float(np.abs(out).mean()))
